# revision 8
# baseline (speedup 1.0000x reference)
"""Trainium2 Bass kernel for the pairwise-alignment CRF loss (nn_CRFLoss).

Strategy
--------
Data parallel: batch 64 -> 8 cores x 8 batches. Per core, the log-domain
wavefront DP is reformulated as a probability-domain row sweep:

    M[i,j] = Em[i,j] * (U[i-1,j-1] + wh)
    X[i,j] = Ex[i,j] *  V[i-1,j]
    Y[i,j] = Ey[i,j] * (W02*M[i,j-1] + W12*X[i,j-1] + W22*Y[i,j-1])
    U = W00*M + W10*X + W20*Y ;  V = W01*M + W11*X + W21*Y

The in-row Y recurrence is a first-order linear scan handled by the DVE
TensorTensorScan instruction. Rows are swept with a skewed wavefront over
15 column chunks of 26 (partition p = b*16 + k holds batch b, chunk k;
slot k=0 is a constant-zero feeder), so chunk k processes row t-(k-1) at
step t and all cross-chunk halos travel one partition per step via a
single stream_shuffle. Each partition carries its own scale sigma_p
(rescaled every REVERY steps, exact cross-chunk ratio correction rho).

v3 highlights vs the f32 baseline:
 - emissions pre-exponentiated on host, shipped bf16 (half DMA, no Exp)
 - emissions are zeroed outside the per-batch (maskX, maskY) box: monotone
   paths through outside cells can never reach a valid end cell, so this
   is exactly equivalent to end-masking -> no gate/mask tensors or ops
 - gold-path cells outside the box (still scored by the reference) are
   stashed in skew slots no DP op reads (cols 0/27 of each 81-col block)
 - cGamma folded into the X-channel bias: V = hp + gp becomes a plain
   TensorTensor, legal on the Pool engine
 - the gp STT emits per-step Z row-sums via accum_out (no zacc pipeline)
 - window rescale factors / Z masses stored raw; all Ln deferred to one
   endgame pass (no activation-table churn in the loop)
 - rescale cadence 20 steps (bf16 exponent range is ample)
"""

import sys

sys.path.insert(0, "/opt/trn_rl_repo")

import numpy as np
import ml_dtypes

# ---------------- fixed problem geometry ----------------
B_FULL, XDIM, YDIM, LPATH = 64, 384, 384, 512
NCORES = 8
BPC = 8                   # batches per core
NK = 16                   # partition slots per batch (slot 0 = zero feeder)
NCH = 15                  # column chunks
CW = 26                   # chunk width (15*26 = 390 >= 384)
NP = BPC * NK             # 128 partitions
TSTEPS = 400              # wavefront steps (384 rows + 15 skew, rounded)
BLK = 16                  # steps per emission block
NBLK = TSTEPS // BLK
REVERY = 20               # rescale / Z-fold cadence
NW = TSTEPS // REVERY     # Z windows
ROWS_P, COLS_P = 416, 418  # padded obs (top 14, left 27)
ROWW = TSTEPS * 81        # skew tensor row width (bf16 elements)

# state tile columns: [U 0:27 | B 27:54 | Y'' 54:81 | V 81:107 | h 107]
US, BS, YS, VS, HC, STW = 0, 27, 54, 81, 107, 108
RESC = 108  # rescaled column range [0:108)

# consts tile columns
C_CA, C_CBETA, C_CBQ, C_TMT, C_H0, C_T9 = range(6)
NCONST = 6

_PROGRAM = None


def _build_program():
    import concourse.bass as bass
    import concourse.bacc as bacc
    import concourse.mybir as mybir
    import concourse.tile as tile

    f32 = mybir.dt.float32
    bf16 = mybir.dt.bfloat16
    i32 = mybir.dt.int32
    Op = mybir.AluOpType
    AF = mybir.ActivationFunctionType

    nc = bacc.Bacc(
        "TRN2",
        target_bir_lowering=False,
        debug=False,
        enable_asserts=False,
        num_devices=NCORES,
    )

    skew = nc.dram_tensor("skew", [NP, ROWW], bf16, kind="ExternalInput")
    consts = nc.dram_tensor("consts", [NP, NCONST], f32, kind="ExternalInput")
    offsd = nc.dram_tensor("offs", [NP, 32], i32, kind="ExternalInput")
    ctrd = nc.dram_tensor("ctr", [16, BPC], f32, kind="ExternalInput")
    identd = nc.dram_tensor("ident", [NP, NP], f32, kind="ExternalInput")
    idt8d = nc.dram_tensor("ind8", [NP, BPC], f32, kind="ExternalInput")
    lossd = nc.dram_tensor("loss", [BPC], f32, kind="ExternalOutput")

    # stream_shuffle mask: within each 16-slot group, slot k reads slot k-1
    # (slot 0 reads itself -> stays zero).
    shmask = [(i if i % NK == 0 else i - 1) for i in range(32)]

    def colap(ap, start, step, count):
        return bass.AP(tensor=ap.tensor, offset=ap.offset + start,
                       ap=[ap.ap[0], [step, count]])

    with tile.TileContext(nc) as tc:
        with (
            tc.tile_pool(name="persist", bufs=1) as pp,
            tc.tile_pool(name="eblk", bufs=3) as ep,
            tc.tile_pool(name="tmp", bufs=6) as wp,
            tc.tile_pool(name="qps", bufs=2, space="PSUM") as qp,
        ):
            st = pp.tile([NP, STW], bf16)
            zsum = pp.tile([NP, TSTEPS], f32)   # per-step Z row-sums
            aw = pp.tile([NP, NW], f32)         # raw window mass
            rpw = pp.tile([NP, NW], f32)        # raw window rescale factors
            cs = pp.tile([NP, NCONST], f32)
            idt = pp.tile([NP, NP], f32)
            idt8 = pp.tile([NP, BPC], f32)
            rho = pp.tile([NP, 1], f32)
            gth = pp.tile([NP, 32], bf16)
            ofs = pp.tile([NP, 32], i32)
            ctr = pp.tile([16, BPC], f32)
            gsum = pp.tile([NP, 1], f32)
            ali8 = pp.tile([BPC, 1], f32)
            one1 = pp.tile([1, 1], f32)

            # ---- loads + init (endgame-only tensors loaded later) ----
            nc.sync.dma_start(out=cs, in_=consts.ap())
            nc.sync.dma_start(out=ofs, in_=offsd.ap())
            nc.vector.memset(st[:, 0:STW], 0.0)
            nc.vector.memset(rpw[:], 1.0)
            nc.vector.memset(rho[:], 1.0)
            nc.vector.memset(one1[:], 1.0)
            nc.vector.tensor_copy(st[:, HC:HC + 1], cs[:, C_H0:C_H0 + 1])

            # ---- alignment-score gathers (overlap with the DP) ----
            # HW indirect DMA: ONE offset per partition per instruction.
            obs_flat = bass.AP(tensor=skew.ap().tensor, offset=0,
                               ap=[[1, NP * ROWW], [1, 1]])
            for w in range(32):
                nc.gpsimd.indirect_dma_start(
                    out=gth[:, w:w + 1], out_offset=None, in_=obs_flat,
                    in_offset=bass.IndirectOffsetOnAxis(ap=ofs[:, w:w + 1], axis=0))

            # ---- main skewed wavefront ----
            h_ap = st[:, HC:HC + 1]
            for blk in range(NBLK):
                t0 = blk * BLK
                et = ep.tile([NP, BLK * 81], bf16, tag="et")
                nc.sync.dma_start(
                    out=et[:], in_=skew.ap()[:, t0 * 81:(t0 + BLK) * 81])

                for tau in range(BLK):
                    t = t0 + tau
                    base = tau * 81
                    em = et[:, base + 1:base + 27]
                    exg = et[:, base + 27 + 1:base + 27 + 27]
                    eys = et[:, base + 54 + 0:base + 54 + 26]
                    eyu = et[:, base + 54 + 1:base + 54 + 27]

                    gp = wp.tile([NP, CW], bf16, tag="gp")
                    hp = wp.tile([NP, CW], bf16, tag="hp")
                    yh = wp.tile([NP, CW], bf16, tag="yh")
                    w1 = wp.tile([NP, CW], bf16, tag="w1")

                    # G' = Em * (U_sh + h); row-sum feeds Z directly
                    nc.vector.scalar_tensor_tensor(
                        gp[:], st[:, US:US + 26], h_ap, em, op0=Op.add, op1=Op.mult,
                        accum_out=zsum[:, t:t + 1])
                    # halo shuffle + rho correction
                    hsrc = colap(st[:], US + 26, 27, 3)
                    hdst = colap(st[:], US, 27, 3)
                    nc.vector.stream_shuffle(hdst, hsrc, shmask)
                    nc.vector.tensor_scalar(out=hdst, in0=hdst,
                                            scalar1=rho[:, 0:1], scalar2=None,
                                            op0=Op.mult)
                    # H2 = (cGamma*Ex) * V_prev  (cGamma folded into channel)
                    nc.gpsimd.tensor_tensor(hp[:], exg, st[:, VS:VS + 26], op=Op.mult)
                    # B2 = (cGamma*cBq)*G' + H2   (= cGamma * B)
                    nc.vector.scalar_tensor_tensor(
                        st[:, BS + 1:BS + 27], gp[:], cs[:, C_CBQ:C_CBQ + 1], hp[:],
                        op0=Op.mult, op1=Op.add)
                    # Y''2 scan (state carries the cGamma scale consistently)
                    nc.vector.tensor_tensor_scan(
                        st[:, YS + 1:YS + 27], eys, st[:, BS:BS + 26],
                        initial=st[:, YS:YS + 1], op0=Op.mult, op1=Op.add)
                    # Yhat2 = Ey' * Y''2   (bf16 TT -> 2x mode)
                    nc.vector.tensor_tensor(yh[:], eyu, st[:, YS + 1:YS + 27], op=Op.mult)
                    # U = (cA/cGamma)*(cBeta*Yhat2 + H2) + G'
                    nc.vector.scalar_tensor_tensor(
                        w1[:], yh[:], cs[:, C_CBETA:C_CBETA + 1], hp[:],
                        op0=Op.mult, op1=Op.add)
                    nc.vector.scalar_tensor_tensor(
                        st[:, US + 1:US + 27], w1[:], cs[:, C_CA:C_CA + 1], gp[:],
                        op0=Op.mult, op1=Op.add)
                    # V = H2 + G'  (plain TT: Pool-legal)
                    nc.gpsimd.tensor_tensor(st[:, VS:VS + 26], hp[:], gp[:], op=Op.add)

                    if t % REVERY == REVERY - 1:
                        wix = t // REVERY
                        rc = wp.tile([NP, 1], f32, tag="rc")
                        rs = wp.tile([NP, 1], f32, tag="rs")
                        rp = rpw[:, wix:wix + 1]
                        # Z window fold: aw[wix] = sum_t zsum[t] over the window
                        nc.vector.reduce_sum(aw[:, wix:wix + 1],
                                             zsum[:, t - (REVERY - 1):t + 1],
                                             axis=mybir.AxisListType.X)
                        # per-partition rescale (raw factor stored, logs deferred)
                        nc.vector.reduce_max(rp, st[:, 0:RESC],
                                             axis=mybir.AxisListType.X)
                        nc.vector.tensor_scalar(out=rp, in0=rp, scalar1=1e-30,
                                                scalar2=None, op0=Op.max)
                        nc.vector.reciprocal(rc[:], rp)
                        # state rescale on the otherwise-idle Act engine
                        nc.scalar.activation(st[:, 0:RESC], st[:, 0:RESC], AF.Copy,
                                             bias=0.0, scale=rc[:, 0:1])
                        # rho *= r_{p-1} / r_p
                        nc.vector.stream_shuffle(rs[:], rp, shmask)
                        nc.vector.tensor_tensor(rs[:], rs[:], rc[:], op=Op.mult)
                        nc.vector.tensor_tensor(rho[:], rho[:], rs[:], op=Op.mult)

            # ---- endgame: Z per batch, then loss ----
            nc.sync.dma_start(out=idt, in_=identd.ap())
            nc.sync.dma_start(out=idt8, in_=idt8d.ap())
            nc.sync.dma_start(out=ctr, in_=ctrd.ap())
            # deferred logs: awlog[w] = ln(aw[w]) + sum_{w'<w} ln(rpw[w'])
            lnrp = pp.tile([NP, NW], f32)
            S = pp.tile([NP, NW], f32)
            zeros = pp.tile([NP, NW], f32)
            awlog = pp.tile([NP, NW], f32)
            fl = pp.tile([NP, NW], f32)
            nc.scalar.activation(lnrp[:], rpw[:], AF.Ln)
            nc.vector.memset(zeros[:], 0.0)
            nc.vector.memset(S[:, 0:1], 0.0)
            nc.vector.tensor_tensor_scan(
                S[:, 1:NW], zeros[:, 1:NW], lnrp[:, 0:NW - 1],
                initial=0.0, op0=Op.add, op1=Op.add)
            nc.vector.tensor_scalar(out=awlog[:], in0=aw[:], scalar1=1.3e-38,
                                    scalar2=None, op0=Op.max)
            nc.scalar.activation(awlog[:], awlog[:], AF.Ln)
            nc.vector.tensor_tensor(awlog[:], awlog[:], S[:], op=Op.add)
            # empty-window guard: aw < 1.4e-38 -> -1e30
            nc.vector.tensor_scalar(out=fl[:], in0=aw[:], scalar1=1.4e-38,
                                    scalar2=None, op0=Op.is_lt)
            nc.vector.scalar_tensor_tensor(
                awlog[:], fl[:], -1.0e30, awlog[:], op0=Op.mult, op1=Op.add)

            # gather -> ln -> per-partition sum (obs part of gold-path score)
            gclamp = pp.tile([NP, 32], bf16)
            lng = pp.tile([NP, 32], f32)
            nc.vector.tensor_scalar(out=gclamp[:], in0=gth[:], scalar1=1e-30,
                                    scalar2=None, op0=Op.max)
            nc.scalar.activation(lng[:], gclamp[:], AF.Ln)
            nc.vector.reduce_sum(gsum[:], lng[:], axis=mybir.AxisListType.X)

            # logsumexp over windows -> apart [NP,1]
            rmax = pp.tile([NP, 1], f32)
            dw = pp.tile([NP, NW], f32)
            sw = pp.tile([NP, 1], f32)
            apart = pp.tile([NP, 1], f32)
            nc.vector.reduce_max(rmax[:], awlog[:], axis=mybir.AxisListType.X)
            nc.vector.tensor_scalar(out=dw[:], in0=awlog[:], scalar1=rmax[:, 0:1],
                                    scalar2=None, op0=Op.subtract)
            nc.scalar.activation(dw[:], dw[:], AF.Exp)
            nc.vector.reduce_sum(sw[:], dw[:], axis=mybir.AxisListType.X)
            nc.scalar.activation(sw[:], sw[:], AF.Ln)
            nc.vector.tensor_tensor(apart[:], sw[:], rmax[:], op=Op.add)

            # cross-slot logsumexp per batch via PE transpose
            pt = qp.tile([1, NP], f32, space="PSUM")
            nc.tensor.matmul(pt[:], apart[:, 0:1], idt[:], start=True, stop=True)
            at = pp.tile([1, NP], f32)
            nc.vector.tensor_copy(at[:], pt[:])
            atv = at[:].rearrange("p (b k) -> p b k", b=BPC, k=NK)
            mb8 = pp.tile([1, BPC], f32)
            s8 = pp.tile([1, BPC], f32)
            nc.vector.reduce_max(mb8[:], atv, axis=mybir.AxisListType.X)
            mb8b = bass.AP(tensor=mb8[:].tensor, offset=mb8[:].offset,
                           ap=[mb8[:].ap[0], [1, BPC], [0, NK]])
            nc.vector.tensor_tensor(atv, atv, mb8b, op=Op.subtract)
            nc.scalar.activation(at[:], at[:], AF.Exp)
            nc.vector.reduce_sum(s8[:], atv, axis=mybir.AxisListType.X)
            nc.scalar.activation(s8[:], s8[:], AF.Ln)
            nc.vector.tensor_tensor(s8[:], s8[:], mb8[:], op=Op.add)

            p8 = qp.tile([BPC, 1], f32, space="PSUM")
            nc.tensor.matmul(p8[:], s8[:], one1[:], start=True, stop=True)
            # ali = per-batch obs gather sum + transition-count @ T9 values
            pa = qp.tile([BPC, 1], f32, space="PSUM")
            nc.tensor.matmul(pa[:], idt8[:], gsum[:], start=True, stop=False)
            nc.tensor.matmul(pa[:], ctr[:], cs[0:16, C_T9:C_T9 + 1],
                             start=False, stop=True)
            nc.vector.tensor_copy(ali8[:], pa[:])
            u8 = pp.tile([BPC, 1], f32)
            nc.vector.scalar_tensor_tensor(u8[:], ali8[:], -1.0, p8[:],
                                           op0=Op.mult, op1=Op.add)
            nc.vector.tensor_scalar(out=u8[:], in0=u8[:],
                                    scalar1=cs[0:BPC, C_TMT:C_TMT + 1],
                                    scalar2=None, op0=Op.add)
            loss_ap = bass.AP(tensor=lossd.ap().tensor, offset=0, ap=[[1, BPC], [1, 1]])
            nc.sync.dma_start(out=loss_ap, in_=u8[:])

    nc.compile()
    return nc


def _get_program():
    global _PROGRAM
    if _PROGRAM is None:
        _PROGRAM = _build_program()
    return _PROGRAM


def _prepare_inputs(observations, trans, P, alignments, maskX, maskY):
    """Host-side marshalling: pad/shard obs, pre-exponentiate, bake offsets."""
    observations = np.asarray(observations, np.float32)
    trans = np.asarray(trans, np.float32)
    P = np.asarray(P, np.float32)
    alignments = np.asarray(alignments).astype(np.int64)
    maskX = np.asarray(maskX).astype(np.int64)
    maskY = np.asarray(maskY).astype(np.int64)

    T = (trans + P).astype(np.float64)
    cA = np.exp(T[1, 0] - T[0, 0])
    cBeta = np.exp(T[2, 0] + T[1, 2] - T[1, 0] - T[2, 2])
    cGamma = np.exp(T[1, 1] - T[0, 1])
    cBq = np.exp(T[0, 2] - T[1, 2])
    h0 = np.exp(T[3, 0] - T[0, 0])
    # channel log-offsets actually stored: M += T00, X += T11 (cGamma
    # pre-folded: exp(x + T01) * cGamma = exp(x + T11)), Y += T22
    bias = np.array([T[0, 0], T[1, 1], T[2, 2]])

    consts = np.zeros((NP, NCONST), np.float32)
    consts[:, C_CA] = cA / cGamma
    consts[:, C_CBETA] = cBeta
    consts[:, C_CBQ] = cBq * cGamma
    consts[:, C_H0] = h0
    T9 = np.zeros(NP, np.float32)
    Tf = (trans + P).astype(np.float32)
    for s_ in range(3):
        for s2 in range(3):
            T9[s_ * 3 + s2] = Tf[s_, s2]
    consts[:, C_T9] = T9

    ident = np.eye(NP, dtype=np.float32)
    ind8 = np.zeros((NP, BPC), np.float32)
    ind8[np.arange(NP), np.arange(NP) // NK] = 1.0

    kidx = np.arange(NP) % NK
    bidx = np.arange(NP) // NK
    # skew gather index grids (shared across cores)
    tg = np.arange(TSTEPS)
    rowg = tg[None, :] - kidx[:, None] + 15          # [NP, T] padded row index
    colg = (CW * kidx)[:, None] + np.arange(27)[None, :]  # [NP, 27] padded col

    in_maps = []
    for c in range(NCORES):
        bs = slice(c * BPC, (c + 1) * BPC)
        ob = observations[bs]
        mXc, mYc = maskX[bs], maskY[bs]
        # pre-exponentiated, channel-biased emission probabilities; zero
        # outside each batch's (maskX, maskY) box (monotone-path argument)
        prob = np.exp(ob + bias[None, None, None, :].astype(np.float32))
        for bb in range(BPC):
            prob[bb, mXc[bb]:, :, :] = 0.0
            prob[bb, :, mYc[bb]:, :] = 0.0
        obp = np.zeros((BPC, ROWS_P, COLS_P, 3), np.float32)
        obp[:, 14:14 + XDIM, 27:27 + YDIM, :] = prob
        # skewed planar emission layout [p, t, s, j]
        sk = obp[bidx[:, None, None, None], rowg[:, :, None, None],
                 colg[:, None, None, :], np.arange(3)[None, None, :, None]]
        skewarr = sk.reshape(NP, ROWW).astype(ml_dtypes.bfloat16)

        al = alignments[bs]
        x1, y1, s1 = al[..., 0], al[..., 1], al[..., 2]
        bloc = np.arange(BPC)[:, None]
        # path point (x,y,s): chunk k = (y-1)//26 + 1, local col = (y-1)%26 + 1,
        # step t = (x-1) + (k-1); element offset into skew[p, t, s, j]
        kk = (y1 - 1) // CW + 1
        jloc = (y1 - 1) % CW + 1
        tt = (x1 - 1) + (kk - 1)
        pp_ = bloc * NK + kk
        flatoff = pp_ * ROWW + (tt * 3 + s1) * 27 + jloc   # [BPC, 512]
        # out-of-box path cells were zeroed above but the reference still
        # scores them: stash exp(obs+bias) in slots no DP op reads (cols
        # 0 and 27 of each 81-block on the zero-feeder partitions)
        oob = (x1 > mXc[:, None]) | (y1 > mYc[:, None])    # [BPC, 512]
        val = np.exp(ob[bloc, x1 - 1, y1 - 1, s1]
                     + bias[s1].astype(np.float32))        # [BPC, 512]
        stash_p = np.arange(BPC) * NK                       # feeder partitions
        si = 0
        for bb in range(BPC):
            for l in np.nonzero(oob[bb])[0]:
                t_s, c_s = divmod(si, 2)
                pfe = stash_p[t_s % BPC]
                tslot = t_s // BPC
                off = pfe * ROWW + tslot * 81 + (0 if c_s == 0 else 27)
                skewarr[pfe, tslot * 81 + (0 if c_s == 0 else 27)] = \
                    ml_dtypes.bfloat16(val[bb, l])
                flatoff[bb, l] = off
                si += 1
        assert si <= 2 * BPC * TSTEPS, "stash overflow"
        # arrange: value (b, l) -> partition b*16 + l%16, column l//16
        offs = np.zeros((NP, 32), np.int32)
        ll = np.arange(LPATH)
        for bb in range(BPC):
            offs[bb * NK + (ll % NK), ll // NK] = flatoff[bb]
        # transition pair counts
        pair = (s1[:, :-1] * 3 + s1[:, 1:]).astype(np.int64)   # [BPC, 511]
        ctr = np.zeros((16, BPC), np.float32)
        for bb in range(BPC):
            cnt = np.bincount(pair[bb], minlength=9)
            ctr[:9, bb] = cnt
        # per-batch endgame constant: t_m_tail + sum of channel biases along
        # the path (gathered values are exp(obs + bias), so their logs
        # overshoot the reference obs-sum by sum(bias[s_l]))
        csb = consts.copy()
        scount = np.stack([(s1 == s_).sum(axis=-1) for s_ in range(3)], axis=-1)
        bias_sum = (scount * bias[None, :]).sum(axis=-1)   # [BPC]
        csb[0:BPC, C_TMT] = (T[0, 4] + bias_sum).astype(np.float32)

        in_maps.append({
            "skew": skewarr,
            "consts": csb,
            "offs": offs,
            "ctr": ctr,
            "ident": ident,
            "ind8": ind8,
        })
    return in_maps


def kernel(observations, trans, P, alignments, maskX, maskY):
    from concourse import bass_utils

    in_maps = _prepare_inputs(observations, trans, P, alignments, maskX, maskY)
    nc = _get_program()
    res = bass_utils.run_bass_kernel_spmd(nc, in_maps, core_ids=list(range(NCORES)))
    out = np.concatenate([np.asarray(res.results[c]["loss"], np.float32)
                          for c in range(NCORES)])
    return out
